# revision 7
# baseline (speedup 1.0000x reference)
"""Trainium2 Bass kernel for nn_DiffMambaLayer (8 NeuronCores, SPMD), v2.

Sharding: 8 cores = (batch b in {0,1}) x (sequence quarter i in {0..3});
each core processes an extended window of LE=3136 tokens (WARM=32 warm-up
tokens on interior edges) so cores are fully independent (no collectives).

v2 structure (vs v1 baseline):
  - conv folded into in_proj on host: xc = silu(sum_k Mk . xn(t+sh) + b)
    with Mk = diag(conv_w_k) @ W_in_xh -> no xh tile, no PSUM->SBUF copy.
  - x_proj/dt_proj folded: dt_pre = (Wdt @ Wxp[:8]) . xc directly; B_n/C_n
    broadcast rows produced by column-replicated stationaries (free-stride-0
    AP over a single stored column) . xc -> no proj tile, no selw.
  - scan payload bf16; dB/hc multiplies read pb/pcr straight from PSUM;
    work split between DVE and Pool (gpsimd, `standard` library only).
  - layernorm partition sums via PE all-ones matmul (no gpsimd allreduce,
    no library switching); activation-table usage phase-batched so the
    Act engine swaps tables ~5x instead of ~81x.
  - y-accumulation over states via identity-matmul PSUM accumulation.
"""
import os
import numpy as np
from contextlib import ExitStack

import concourse.bacc as bacc
import concourse.bass as bass
import concourse.mybir as mybir
from concourse import tile, bass_utils, library_config
import bass_rust as _br
import ml_dtypes

F32 = mybir.dt.float32
F32R = mybir.dt.float32r
BF16 = mybir.dt.bfloat16
AF = mybir.ActivationFunctionType
OP = mybir.AluOpType

B, C, T, HH, WW = 2, 128, 48, 16, 16
L = T * HH * WW            # 12288
LSH = L // 4               # 3072
WARM = 32
LE = LSH + 2 * WARM        # 3136
N = 8
EPS = 1e-5
EXT_LO = [0, LSH - WARM, 2 * LSH - WARM, 3 * LSH - 2 * WARM]
OFF = [0, WARM, WARM, 2 * WARM]

CH = [(i * 512, min(512, LE - i * 512)) for i in range((LE + 511) // 512)]
_SW = [256] + [480] * 6
SEG = [(sum(_SW[:i]), _SW[i]) for i in range(7)]

# engine split knobs (gpsimd cannot touch PSUM on real HW: pool multiplies
# need an Act-engine PSUM->SBUF copy first)
HC_ACTPOOL = {1, 3, 5, 7}   # hc via Act-copy + Pool-mult
DB_ACTPOOL = {2, 4, 6}         # dB via Act-copy + Pool-mult
SCAN_POOL = set()           # scans are DVE-only (Pool lacks the opcode)


def _col(t, j):
    return t[:, j:j + 1]


def emit(nc, tc, ctx, dr, trivial_ln):
    full = ctx.enter_context(tc.tile_pool(name="full", bufs=1))
    scan2 = ctx.enter_context(tc.tile_pool(name="scan2", bufs=3))
    cst = ctx.enter_context(tc.tile_pool(name="cst", bufs=1))
    pmain = ctx.enter_context(tc.tile_pool(name="pmain", bufs=2, space="PSUM"))
    prep = ctx.enter_context(tc.tile_pool(name="prep", bufs=2, space="PSUM"))
    pyy = ctx.enter_context(tc.tile_pool(name="pyy", bufs=1, space="PSUM"))

    # one gpsimd library for the whole kernel (plain tensor ops only)
    libgate = nc.gpsimd.load_library(library_config.standard)

    def gp(ins):
        _br.add_dep_helper(ins.ins, libgate.ins, sync=False,
                           reason="gpsimd library ordering")
        return ins

    # Pin the Act engine's stream to emission order: the scheduler otherwise
    # interleaves Silu-set and Exp-set ops, reloading the activation table
    # (1283ns) ~90x. With emission-order phases it loads ~5x.
    act_prev = [None]

    def _achain(ins):
        if act_prev[0] is not None:
            _br.add_dep_helper(ins.ins, act_prev[0].ins, sync=False,
                               reason="act table phase order")
        act_prev[0] = ins
        return ins

    class _ActWrap:
        def activation(self, *a, **k):
            return _achain(nc.scalar.activation(*a, **k))

        def copy(self, *a, **k):
            return _achain(nc.scalar.copy(*a, **k))

    acte = _ActWrap()

    xs = full.tile([128, LE], F32R, tag="xs")

    # ---- params to SBUF ----
    wconv = cst.tile([128, 2048], F32R, tag="wconv")   # [c, u*512 + k*128 + c']
    wz = cst.tile([128, 256], F32R, tag="wz")          # [c, m*128 + c']
    wdtx = cst.tile([128, 512], BF16, tag="wdtx")      # [c, u*128 + c']
    wbc = cst.tile([128, 8192], BF16, tag="wbc")       # [c, (u*16+r*8+n)*128 + o]
    wout = cst.tile([128, 256], BF16, tag="wout")      # [d, m*128 + c']
    wdiag = cst.tile([128, 512], BF16, tag="wdiag")    # diag(Dp) per u
    ones32 = cst.tile([128, 128], F32, tag="ones")
    ident = cst.tile([128, 128], BF16, tag="ident")
    dtb = cst.tile([128, 4], F32, tag="dtb")
    Aw = cst.tile([128, 32], F32, tag="Aw")            # [d, u*8 + n]
    cvb = cst.tile([128, 4], F32, tag="cvb")
    lnp = cst.tile([128, 8], F32, tag="lnp")

    # tiny LN params first, then the x slice (LN1 gates everything), then
    # the rest of the weights
    nc.sync.dma_start(lnp[:], dr["lnp"][:])
    nc.sync.dma_start(ones32[:], dr["ones"].bitcast(F32)[:])
    for cs, cl in CH:
        nc.sync.dma_start(xs[:, cs:cs + cl],
                          dr["xs"].bitcast(F32R)[:, cs:cs + cl])
    nc.sync.dma_start(dtb[:], dr["dtb"][:])
    nc.sync.dma_start(Aw[:], dr["Aw"][:])
    nc.sync.dma_start(cvb[:], dr["cvb"][:])
    nc.sync.dma_start(wconv[:], dr["wconv"][:])
    nc.sync.dma_start(wz[:], dr["wz"][:])
    nc.sync.dma_start(wdtx[:], dr["wdtx"][:])
    nc.sync.dma_start(wdiag[:], dr["wdiag"][:])
    nc.sync.dma_start(ident[:], dr["ident"][:])
    nc.sync.dma_start(wout[:], dr["wout"][:])
    nc.sync.dma_start(wbc[:], dr["wbc"][:])

    # PE p-state warmup: ~4us of dummy matmuls so LN1/conv hit full clock
    warm = pmain.tile([128, 128], F32, tag="mm", name="warmup", bufs=1)
    for w in range(8):
        nc.tensor.matmul(warm[:], ones32[:], ones32[:],
                         start=(w == 0), stop=(w == 7))

    def wbc_st(u, r, n):
        j = (u * 16 + r * 8 + n) * 128
        return wbc[:, j:j + 128]


    def layernorm(x_el, wj, bj, out_ap, var, xm, out_act=False):
        """out = (x - mean_c(x)) * rsqrt(var_c + eps) [* w + b] (partition LN).

        Partition sums via all-ones fp32 PE matmul (every output partition
        gets the sum); squares on the otherwise-idle Pool engine. out_act
        forces the final write through Act so the result is fp32r-rounded
        for downstream fp32r matmuls."""
        for cs, cl in CH:
            p1 = prep.tile([128, cl], F32, tag="repb", bufs=2,
                           name=f"lns1_{wj}_{cs}")
            nc.tensor.matmul(p1[:], ones32[:], x_el[:, cs:cs + cl],
                             start=True, stop=True)
            nc.vector.scalar_tensor_tensor(xm.bitcast(F32)[:, cs:cs + cl],
                                           p1[:], -1.0 / 128,
                                           x_el[:, cs:cs + cl],
                                           OP.mult, OP.add)
            gp(nc.gpsimd.tensor_mul(var.bitcast(F32)[:, cs:cs + cl],
                                    xm.bitcast(F32)[:, cs:cs + cl],
                                    xm.bitcast(F32)[:, cs:cs + cl]))
            sq = pyy.tile([128, cl], F32, tag="y", bufs=2,
                          name=f"lnsq_{wj}_{cs}")
            nc.tensor.matmul(sq[:], ones32[:],
                             var.bitcast(F32)[:, cs:cs + cl],
                             start=True, stop=True)
            acte.activation(var.bitcast(F32)[:, cs:cs + cl], sq[:],
                            AF.Sqrt, scale=1.0 / 128, bias=_col(lnp, 6))
        for cs, cl in CH:
            nc.vector.reciprocal(var.bitcast(F32)[:, cs:cs + cl],
                                 var.bitcast(F32)[:, cs:cs + cl])   # rstd
            if trivial_ln and not out_act:
                eng = nc.vector if (cs // 512) % 2 == 0 else None
                if eng is not None:
                    eng.tensor_mul(out_ap[:, cs:cs + cl],
                                   xm.bitcast(F32)[:, cs:cs + cl],
                                   var.bitcast(F32)[:, cs:cs + cl])
                else:
                    gp(nc.gpsimd.tensor_mul(out_ap[:, cs:cs + cl],
                                            xm.bitcast(F32)[:, cs:cs + cl],
                                            var.bitcast(F32)[:, cs:cs + cl]))
            else:
                gp(nc.gpsimd.tensor_mul(xm.bitcast(F32)[:, cs:cs + cl],
                                        xm.bitcast(F32)[:, cs:cs + cl],
                                        var.bitcast(F32)[:, cs:cs + cl]))
                if trivial_ln:
                    acte.activation(out_ap[:, cs:cs + cl],
                                    xm.bitcast(F32)[:, cs:cs + cl],
                                    AF.Identity)
                else:
                    acte.activation(out_ap[:, cs:cs + cl],
                                    xm.bitcast(F32)[:, cs:cs + cl],
                                    AF.Identity,
                                    bias=_col(lnp, bj), scale=_col(lnp, wj))

    # ---- LN1 -> xn (padded by 3 zero cols each side) ----
    xn = full.tile([128, LE + 6], F32R, tag="xn")
    var = full.tile([128, LE], F32R, tag="var")
    xm = full.tile([128, LE], F32R, tag="xm")
    acte.activation(xn[:, 0:3], lnp[:, 0:3], AF.Identity, scale=0.0)
    acte.activation(xn[:, LE + 3:LE + 6], lnp[:, 0:3], AF.Identity,
                    scale=0.0)
    layernorm(xs.bitcast(F32)[:], 0, 1, xn[:, 3:LE + 3], var, xm,
              out_act=True)

    attn = full.tile([128, LE], F32R, tag="attn")

    xcs, dts, wins, szs = {}, {}, {}, {}

    def conv_block(m, d):
        u = 2 * m + d
        xc = full.tile([128, LE], BF16, tag="xc", name=f"xc{u}", bufs=3)
        xcs[u] = xc
        for cs, cl in CH:
            pc = pmain.tile([128, cl], F32,
                            tag=("mm" if (cs // 512) % 2 == 0 else "mmz"),
                            name=f"pc{u}_{cs}", bufs=1)
            for k in range(4):
                sh = (k - 3) if d == 0 else (3 - k)
                nc.tensor.matmul(
                    pc[:], wconv[:, u * 512 + k * 128:u * 512 + (k + 1) * 128],
                    xn[:, 3 + cs + sh:3 + cs + sh + cl],
                    start=(k == 0), stop=(k == 3))
            acte.activation(xc[:, cs:cs + cl], pc[:],
                            AF.Silu, bias=_col(cvb, u))

    def z_block(m):
        sz = full.tile([128, LE], BF16, tag="sz", name=f"sz{m}", bufs=2)
        szs[m] = sz
        for cs, cl in CH:
            pz = pmain.tile([128, cl], F32,
                            tag=("mm" if (cs // 512) % 2 == 0 else "mmz"),
                            name=f"pz{m}_{cs}", bufs=1)
            nc.tensor.matmul(pz[:], wz[:, m * 128:(m + 1) * 128],
                             xn[:, 3 + cs:3 + cs + cl], start=True, stop=True)
            acte.activation(sz[:, cs:cs + cl], pz[:], AF.Silu)

    def dt_block(m, d):
        u = 2 * m + d
        xc = xcs[u]
        dt = full.tile([128, LE], BF16, tag="dt", name=f"dt{u}", bufs=2)
        win = full.tile([128, LE], BF16, tag="win", name=f"win{u}", bufs=2)
        dts[u], wins[u] = dt, win
        for cs, cl in CH:
            pd = pmain.tile([128, cl], F32,
                            tag=("mm" if (cs // 512) % 2 == 0 else "mmz"),
                            name=f"pd{u}_{cs}", bufs=1)
            nc.tensor.matmul(pd[:], wdtx[:, u * 128:(u + 1) * 128],
                             xc[:, cs:cs + cl], start=True, stop=True)
            acte.activation(dt[:, cs:cs + cl], pd[:], AF.Exp,
                            bias=_col(dtb, u))
        for cs in (0, 1568):
            acte.activation(dt[:, cs:cs + 1568], dt[:, cs:cs + 1568],
                            AF.Ln, bias=_col(lnp, 7))
        nc.vector.tensor_mul(win[:], dt[:], xc[:])

    hprev = {}
    dAs_pair = {}

    def seg_block(m, d, si, sgi, first_unit):
        u = 2 * m + d
        xc, dt, win, sz = xcs[u], dts[u], wins[u], szs[m]
        ss, sl = SEG[sgi]
        yps = pyy.tile([128, sl], F32, tag="y", bufs=2, name=f"yps{u}_{ss}")
        hs, hcs = {}, {}
        # Dp * xc folded into the PSUM accumulation (gate g1 becomes free)
        nc.tensor.matmul(yps[:], wdiag[:, u * 128:(u + 1) * 128],
                         xc[:, ss:ss + sl], start=True, stop=False)

        def do_hc(n):
            h = hs.pop(n)
            hc = scan2.tile([128, sl], BF16, tag="hC",
                            name=f"hc{u}_{ss}_{n}")
            hcs[n] = hc
            pcr = prep.tile([128, sl], F32, tag="repc", bufs=2,
                            name=f"pcr{u}_{ss}_{n}")
            nc.tensor.matmul(pcr[:], wbc_st(u, 1, n), xc[:, ss:ss + sl],
                             start=True, stop=True)
            if n in HC_ACTPOOL:
                pcr16 = scan2.tile([128, sl], BF16, tag="pc16", bufs=3,
                                   name=f"pcr16{u}_{ss}_{n}")
                nc.scalar.copy(pcr16[:], pcr[:])
                gp(nc.gpsimd.tensor_mul(hc[:], h[:], pcr16[:]))
            else:
                nc.vector.tensor_mul(hc[:], h[:], pcr[:])

        def yacc(n):
            hc = hcs.pop(n)
            nc.tensor.matmul(yps[:], ident[:], hc[:],
                             start=False, stop=(n == 7))

        for n in range(N):
            nseg = len(SEG)
            grp = (si // 2) * 2
            if si % 2 == 0:
                gsgis = [(s if d == 0 else nseg - 1 - s)
                         for s in range(grp, min(grp + 2, nseg))]
                ds0 = min(SEG[s][0] for s in gsgis)
                dl = sum(SEG[s][1] for s in gsgis)
                dA2 = scan2.tile([128, 960], BF16, tag="dA", bufs=10,
                                 name=f"dA{u}_{si}_{n}")
                acte.activation(dA2[:, :dl], dt[:, ds0:ds0 + dl], AF.Exp,
                                scale=_col(Aw, u * 8 + n))
                dAs_pair.setdefault(n, {})[si] = (dA2, ds0, dl)
            dA2, ds0, dl = dAs_pair[n][grp]
            dA = dA2[:, ss - ds0:ss - ds0 + sl]
            pb = prep.tile([128, sl], F32, tag="repb", bufs=2,
                           name=f"pb{u}_{ss}_{n}")
            nc.tensor.matmul(pb[:], wbc_st(u, 0, n), xc[:, ss:ss + sl],
                             start=True, stop=True)
            dB = scan2.tile([128, sl], BF16, tag="dB",
                            name=f"dB{u}_{ss}_{n}")
            if n in DB_ACTPOOL:
                pb16 = scan2.tile([128, sl], BF16, tag="pb16", bufs=3,
                                  name=f"pb16{u}_{ss}_{n}")
                nc.scalar.copy(pb16[:], pb[:])
                gp(nc.gpsimd.tensor_mul(dB[:], win[:, ss:ss + sl], pb16[:]))
            else:
                nc.vector.tensor_mul(dB[:], win[:, ss:ss + sl], pb[:])
            h = scan2.tile([128, sl], BF16, tag="h", bufs=10,
                           name=f"h{u}_{ss}_{n}")
            if si == 0:
                init = 0.0
            else:
                hp, pl = hprev[n]
                init = hp[:, pl - 1:pl] if d == 0 else hp[:, 0:1]
            hs[n] = h
            hprev[n] = (h, sl)
            seng = nc.gpsimd if n in SCAN_POOL else nc.vector
            if d == 0:
                seng.tensor_tensor_scan(
                    h[:], dA, dB[:], init, OP.mult, OP.add)
            else:
                seng.tensor_tensor_scan(
                    h[:, ::-1], dA[:, ::-1], dB[:, ::-1], init,
                    OP.mult, OP.add)
            if n >= 1:
                do_hc(n - 1)
            if n >= 2:
                yacc(n - 2)
        do_hc(N - 1)
        yacc(N - 2)
        yacc(N - 1)
        # ---- gate + out_proj + attn accumulate ----
        g2 = scan2.tile([128, sl], BF16, tag="g2", name=f"g2{u}_{ss}",
                        bufs=2)
        nc.vector.tensor_mul(g2[:], sz[:, ss:ss + sl], yps[:])
        po = prep.tile([128, sl], F32, tag="repc", bufs=2,
                       name=f"po{u}_{ss}")
        nc.tensor.matmul(po[:], wout[:, m * 128:(m + 1) * 128],
                         g2[:], start=True, stop=True)
        if first_unit:
            nc.vector.tensor_copy(attn[:, ss:ss + sl], po[:])
        else:
            nc.vector.tensor_add(attn[:, ss:ss + sl],
                                 attn.bitcast(F32)[:, ss:ss + sl], po[:])

    # pipeline: u0 (m0,d0), u1 (m0,d1), u3 (m1,d1), u2 (m1,d0);
    # next unit's preproc blocks are emitted between the current unit's
    # segments (set-coherent Act blocks) so no engine drains at boundaries
    conv_block(0, 0)
    dt_block(0, 0)
    z_block(0)
    seq = [(0, 0), (0, 1), (1, 1), (1, 0)]
    pre_work = {
        (0, 1): lambda: conv_block(0, 1),
        (0, 3): lambda: dt_block(0, 1),
        (1, 1): lambda: conv_block(1, 1),
        (1, 2): lambda: (conv_block(1, 0), z_block(1)),
        (1, 3): lambda: dt_block(1, 1),
        (2, 3): lambda: dt_block(1, 0),
    }
    for pi, (m, d) in enumerate(seq):
        nseg = len(SEG)
        order = range(nseg) if d == 0 else range(nseg - 1, -1, -1)
        for si, sgi in enumerate(order):
            w = pre_work.get((pi, si))
            if w is not None:
                w()
            seg_block(m, d, si, sgi, first_unit=(pi == 0))

    # ---- subln(attn), residual, LN2 ----
    layernorm(attn.bitcast(F32)[:], 2, 3, attn.bitcast(F32)[:], var, xm)
    for i in range(4):
        cs, cl = i * 784, 784
        if i % 2 == 0:
            nc.vector.tensor_add(attn[:, cs:cs + cl],
                                 attn.bitcast(F32)[:, cs:cs + cl],
                                 xs.bitcast(F32)[:, cs:cs + cl])
        else:
            gp(nc.gpsimd.tensor_add(attn[:, cs:cs + cl],
                                    attn.bitcast(F32)[:, cs:cs + cl],
                                    xs.bitcast(F32)[:, cs:cs + cl]))
    osb = full.tile([128, LE], F32, tag="xn")
    layernorm(attn.bitcast(F32)[:], 4, 5, osb[:], var, xm)
    for cs, cl in CH:
        nc.sync.dma_start(dr["o"][:, cs:cs + cl], osb[:, cs:cs + cl])


_CACHE = {}
_LAST_RES = None


def _build(trivial_ln=True):
    key = ("nc", trivial_ln)
    if key in _CACHE:
        return _CACHE[key]
    nc = bacc.Bacc("TRN2", target_bir_lowering=False, debug=False,
                   num_devices=8)
    dr = {}
    dr["xs"] = nc.dram_tensor("xs", [128, LE], F32, kind="ExternalInput").ap()
    dr["wconv"] = nc.dram_tensor("wconv", [128, 2048], F32R, kind="ExternalInput").ap()
    dr["wz"] = nc.dram_tensor("wz", [128, 256], F32R, kind="ExternalInput").ap()
    dr["wdtx"] = nc.dram_tensor("wdtx", [128, 512], BF16, kind="ExternalInput").ap()
    dr["wbc"] = nc.dram_tensor("wbc", [128, 8192], BF16, kind="ExternalInput").ap()
    dr["wout"] = nc.dram_tensor("wout", [128, 256], BF16, kind="ExternalInput").ap()
    dr["wdiag"] = nc.dram_tensor("wdiag", [128, 512], BF16, kind="ExternalInput").ap()
    dr["ones"] = nc.dram_tensor("ones", [128, 128], F32R, kind="ExternalInput").ap()
    dr["ident"] = nc.dram_tensor("ident", [128, 128], BF16, kind="ExternalInput").ap()
    dr["dtb"] = nc.dram_tensor("dtb", [128, 4], F32, kind="ExternalInput").ap()
    dr["Aw"] = nc.dram_tensor("Aw", [128, 32], F32, kind="ExternalInput").ap()
    dr["Dp"] = nc.dram_tensor("Dp", [128, 4], F32, kind="ExternalInput").ap()
    dr["cvb"] = nc.dram_tensor("cvb", [128, 4], F32, kind="ExternalInput").ap()
    dr["lnp"] = nc.dram_tensor("lnp", [128, 8], F32, kind="ExternalInput").ap()
    dr["o"] = nc.dram_tensor("o", [128, LE], F32, kind="ExternalOutput").ap()

    with tile.TileContext(nc) as tc:
        with ExitStack() as ctx:
            emit(nc, tc, ctx, dr, trivial_ln)
    nc.compile()
    _CACHE[key] = (nc, dr)
    return nc, dr


def _host_prep(inp):
    f = np.float32
    lam = 1.0 / (1.0 + np.exp(-np.sum(inp["lambda_q"], dtype=np.float64)))
    W_out = np.stack([inp["out_proj_w"][0],
                      -np.float32(lam) * inp["out_proj_w"][1]]).astype(f)
    p = {}
    # wconv[c, u*512 + k*128 + c'] = in_proj_w[m][c', c] * conv_w[m,d,c',k]
    wconv = np.zeros((128, 2048), f)
    for m in range(2):
        win_xh = inp["in_proj_w"][m][:128, :]          # [c_out, c_in]
        for d in range(2):
            u = 2 * m + d
            for k in range(4):
                blk = win_xh.T * inp["conv_w"][m, d, :, k][None, :]
                wconv[:, u * 512 + k * 128:(u * 512) + (k + 1) * 128] = blk
    p["wconv"] = wconv
    wz = np.zeros((128, 256), f)
    for m in range(2):
        wz[:, m * 128:(m + 1) * 128] = inp["in_proj_w"][m][128:, :].T
    p["wz"] = wz
    wdtx = np.zeros((128, 512), f)
    wbc = np.zeros((128, 8192), f)
    for m in range(2):
        for d in range(2):
            u = 2 * m + d
            xp = inp["x_proj_w"][m, d]                 # [24, 128]
            wdtx[:, u * 128:(u + 1) * 128] = (inp["dt_proj_w"][m, d] @ xp[:8, :]).T
            for n in range(8):
                wbc[:, (u * 16 + n) * 128:(u * 16 + n + 1) * 128] = \
                    xp[8 + n, :][:, None]
                j = (u * 16 + 8 + n) * 128
                wbc[:, j:j + 128] = xp[16 + n, :][:, None]
    p["wdtx"] = wdtx.astype(ml_dtypes.bfloat16)
    p["wbc"] = wbc.astype(ml_dtypes.bfloat16)
    wout = np.zeros((128, 256), f)
    for m in range(2):
        wout[:, m * 128:(m + 1) * 128] = W_out[m].T
    p["wout"] = wout.astype(ml_dtypes.bfloat16)
    wdiag = np.zeros((128, 512), f)
    Dr = inp["D"].astype(f).reshape(4, 128)
    for u in range(4):
        np.fill_diagonal(wdiag[:, u * 128:(u + 1) * 128], Dr[u])
    p["wdiag"] = wdiag.astype(ml_dtypes.bfloat16)
    p["ones"] = np.ones((128, 128), f)
    p["ident"] = np.eye(128, dtype=f).astype(ml_dtypes.bfloat16)
    p["dtb"] = np.ascontiguousarray(
        inp["dt_proj_b"].astype(f).reshape(4, 128).T)
    p["Aw"] = np.ascontiguousarray(
        (-np.exp(inp["A_log"])).astype(f).reshape(4, 128, 8)
        .transpose(1, 0, 2).reshape(128, 32))
    p["Dp"] = np.ascontiguousarray(inp["D"].astype(f).reshape(4, 128).T)
    p["cvb"] = np.ascontiguousarray(inp["conv_b"].astype(f).reshape(4, 128).T)
    lnp = np.stack([inp["norm1_w"], inp["norm1_b"], inp["subln_w"],
                    inp["subln_b"], inp["norm2_w"], inp["norm2_b"],
                    np.full(128, EPS), np.ones(128)],
                   axis=1).astype(f)
    p["lnp"] = lnp
    return p


def kernel(**inputs):
    inp = {k: np.asarray(v) for k, v in inputs.items()}
    trivial_ln = all(
        np.all(inp[k + "_w"] == 1.0) and np.all(inp[k + "_b"] == 0.0)
        for k in ("norm1", "norm2", "subln"))
    nc, dr = _build(trivial_ln)
    p = _host_prep(inp)
    x = inp["x"].astype(np.float32).reshape(B, C, L)
    in_maps = []
    for core in range(8):
        b, i = core // 4, core % 4
        m = dict(p)
        m["xs"] = np.ascontiguousarray(x[b, :, EXT_LO[i]:EXT_LO[i] + LE])
        in_maps.append(m)
    trace = bool(os.environ.get("DIFFMAMBA_TRACE"))
    res = bass_utils.run_bass_kernel_spmd(
        nc, in_maps, core_ids=list(range(8)), trace=trace,
        trace_cores=[0] if trace else None)
    global _LAST_RES
    _LAST_RES = res
    out = np.empty((B, C, L), np.float32)
    for core in range(8):
        b, i = core // 4, core % 4
        out[b, :, i * LSH:(i + 1) * LSH] = \
            res.results[core]["o"][:, OFF[i]:OFF[i] + LSH]
    return out.reshape(B, C, T, HH, WW)
